# revision 19
# baseline (speedup 1.0000x reference)
"""TRN2 Bass kernel for nn_MetaBaseline (DN4-style local-descriptor kNN).

Reference computation (per batch b):
  q = input1[b].reshape(7500, 640)           # query patch descriptors
  s = normalize(input2[b].reshape(2500, 640), axis=-1)
  scores = q @ s.T                           # [7500, 2500]
  per way group g (columns [500g, 500g+500)): top-k per row, mean,
  divide by |q_patch| (commutes with top-k since it is per-row), then
  sum over the 100 patches of each query -> out [75, 5].

Sharding: data-parallel over (b, query-quarter): 8 cores, each handles one
batch's quarter of queries (19 queries padded) with that batch's full
support replicated.

Implementation highlights vs the fp32r baseline:
- Score matmuls run in fp8e4m3 with MatmulPerfMode.DoubleRow (two 128-row
  k-tiles per pass): C=640 is padded to 6 chunks of 128 (band 5 zeros) so
  each (patch-tile, way) pair needs just 3 PE instructions.
- The query operand is uploaded pre-transposed in fp8 from the host
  (layout + dtype conversion only), so queries need no PE transpose, no
  PSUM eviction and no quantize pass on device. A second row-major bf16
  copy of q feeds the |q_patch| norm pass (ACT square+accum), matching the
  reference's exact-norm division.
- Support arrives row-major bf16; on device: ACT square+accum -> batched
  sqrt(scale=1/alpha^2) -> DVE reciprocal gives sinv = alpha/|s| per
  descriptor; DVE tensor_scalar (4x mode on bf16) or ACT scales to
  alpha-normalized bf16; PE transposes (bf16 identity, 1 cycle/row) into
  one PSUM bank per tile; a single strided copy evicts+converts to the
  fp8 banded layout. alpha=32 keeps e4m3 operands in the normal range;
  1/alpha folds into the per-query scale sqrt(k^2 alpha^2 sum q^2).
- Top-8 per (patch, way) via DVE max8 straight from the PSUM score bank;
  pass 4 finishes each patch tile with a strided top-k tensor_reduce, an
  ACT scale by 1/(k*alpha*|q_patch|), and a small fp32 indicator matmul
  accumulating per-query sums in PSUM -> [19, 5].
"""
import os
from contextlib import ExitStack

import ml_dtypes
import numpy as np

import concourse.bass as bass  # noqa: F401
import concourse.mybir as mybir
import concourse.tile as tile
from concourse import bacc
from concourse.bass_utils import run_bass_kernel_spmd

# Problem geometry (hardcoded per contest rules)
B, Q, WAY, SHOT, H, W, C = 2, 75, 5, 5, 10, 10, 640
HW = H * W               # 100 patches per query / support image
NQ = 19                  # queries per core (4 cores x 19 = 76 >= 75)
MT = 15                  # patch M-tiles of 128 -> 1920 rows (1900 real)
PAD_P = MT * 128
NS = WAY * SHOT * HW     # 2500 support descriptors per batch
ST = 20                  # support tiles of 128 -> 2560 rows
PAD_S = ST * 128
KC = 5                   # real C chunks of 128 (640 = 5*128)
KP = 3                   # DoubleRow chunk pairs (6 bands incl. zero band)
P = 128
NW = SHOT * HW           # 500 support descriptors per way group
N_CORES = 8
ALPHA = 32.0             # support operand scale for fp8 normal range
N_WARM = int(os.environ.get("N_WARM", "36"))
# NOTE: tensor_tensor_reduce with bf16 inputs faults on real TRN2 hardware
# (verified by bisection); squares therefore run on ACT only.
SQ_DVE = os.environ.get("SQ_DVE", "0") == "1"   # squares split ACT/DVE
TPP_BUFS = int(os.environ.get("TPP", "3"))       # transpose PSUM banks
SCB = os.environ.get("SCB", "1") == "1"          # scaled on DVE bf16 + bf16 ind

_prog_cache: dict[int, object] = {}


def _build(k: int):
    """Build + compile the per-core SPMD program for neighbor_k == k."""
    assert 1 <= k <= 8, f"neighbor_k={k} not supported (need 1..8)"
    nc = bacc.Bacc("TRN2", target_bir_lowering=False, debug=False)
    f32 = mybir.dt.float32
    bf16 = mybir.dt.bfloat16
    fp8 = mybir.dt.float8e4
    AF = mybir.ActivationFunctionType
    DR = mybir.MatmulPerfMode.DoubleRow

    qT_d = nc.dram_tensor("qT", [P, 2 * KP * PAD_P], fp8,
                          kind="ExternalInput").ap()
    qr_d = nc.dram_tensor("qr", [P, MT * C], bf16, kind="ExternalInput").ap()
    sr_d = nc.dram_tensor("sr", [P, ST * C], bf16, kind="ExternalInput").ap()
    zs_d = nc.dram_tensor("zs", [P, PAD_S], fp8, kind="ExternalInput").ap()
    ind_dt = bf16 if SCB else f32
    ind_d = nc.dram_tensor("ind", [P, MT * NQ], ind_dt,
                           kind="ExternalInput").ap()
    ident_d = nc.dram_tensor("ident", [P, P], bf16, kind="ExternalInput").ap()
    out_d = nc.dram_tensor("out", [NQ, WAY], f32, kind="ExternalOutput").ap()

    with tile.TileContext(nc) as tc:
        with ExitStack() as ctx:
            const = ctx.enter_context(tc.tile_pool(name="const", bufs=1))
            big = ctx.enter_context(tc.tile_pool(name="big", bufs=1))
            loads = ctx.enter_context(tc.tile_pool(name="loads", bufs=8))
            work = ctx.enter_context(tc.tile_pool(name="work", bufs=3))
            small = ctx.enter_context(tc.tile_pool(name="small", bufs=4))
            mxp = ctx.enter_context(tc.tile_pool(name="mxp", bufs=MT))
            outp = ctx.enter_context(
                tc.tile_pool(name="outp", bufs=1, space="PSUM")
            )
            tpp = ctx.enter_context(
                tc.tile_pool(name="tpp", bufs=TPP_BUFS, space="PSUM")
            )
            warmp = ctx.enter_context(
                tc.tile_pool(name="warmp", bufs=1, space="PSUM")
            )
            spp = ctx.enter_context(
                tc.tile_pool(name="spp", bufs=3, space="PSUM")
            )

            ident = const.tile([P, P], bf16)
            ind_sb = const.tile([P, MT * NQ], ind_dt)
            # banded transposed operands: [partition, chunk band, column]
            s8T = big.tile([P, 2 * KP, PAD_S], fp8, name="s8T")
            qT_sb = big.tile([P, 2 * KP, PAD_P], fp8, name="qT_sb")
            qr_sb = big.tile([P, MT * C], bf16, name="qr_sb")

            # batched norm scalars
            ssum_all = const.tile([P, ST], f32, name="ssum")
            sinv_all = const.tile([P, ST], f32, name="sinv")
            qsum_all = const.tile([P, MT], f32, name="qsum")
            qinv = const.tile([P, MT], f32, name="qinv")

            out_ps = outp.tile([NQ, WAY], f32)

            # ---- warmups: ACT tables + PE pipeline + pstate ramp ----
            wtile = const.tile([P, P], bf16, name="wtile")
            nc.vector.memset(wtile, 1.0)
            wsq = small.tile([P, P], bf16, tag="wsq")
            wss = small.tile([P, 1], f32, tag="snrm")
            nc.scalar.activation(wsq, wtile, AF.Square, accum_out=wss)
            nc.scalar.sqrt(wss, wss)
            wps = warmp.tile([P, 4 * P], f32, tag="warm")
            for i in range(N_WARM):
                nc.tensor.matmul(
                    wps[:, (i % 4) * P:(i % 4 + 1) * P], wtile, wtile,
                    start=True, stop=True)

            # ---- prologue DMAs ----
            xs_s = [None] * ST

            def s_dma_pair(t):
                # one DMA brings two adjacent support tiles (fewer SP-queue
                # instructions; each costs ~650ns of serial sync-engine time)
                x = loads.tile([P, 2 * C], bf16, tag="x_tile", name=f"sx{t}")
                nc.sync.dma_start(out=x, in_=sr_d[:, t * C:(t + 2) * C])
                xs_s[t] = x[:, :C]
                xs_s[t + 1] = x[:, C:]

            s_dma_pair(0)
            s_dma_pair(2)
            nc.sync.dma_start(out=ident, in_=ident_d)
            nc.sync.dma_start(out=qT_sb, in_=qT_d)
            nc.sync.dma_start(out=s8T[:, 2 * KP - 1, :], in_=zs_d)
            nc.sync.dma_start(out=ind_sb, in_=ind_d)
            nc.sync.dma_start(out=qr_sb, in_=qr_d)

            next_s = [4]

            def s_dma_ahead(upto):
                while next_s[0] <= min(upto, ST - 1):
                    s_dma_pair(next_s[0])
                    next_s[0] += 2

            # ---- prep helpers ----
            def s_sq(t, on_dve=False):
                if on_dve and SQ_DVE:
                    sqf = work.tile([P, C], f32, tag="sqf")
                    nc.vector.tensor_tensor_reduce(
                        sqf, xs_s[t], xs_s[t], 1.0, 0.0,
                        mybir.AluOpType.mult, mybir.AluOpType.add,
                        ssum_all[:, t:t + 1])
                else:
                    sq = work.tile([P, C], bf16, tag="sq")
                    nc.scalar.activation(sq, xs_s[t], AF.Square,
                                         accum_out=ssum_all[:, t:t + 1])

            def s_finish(t0, n):
                # tiles t0..t0+n-1: snrm = sqrt(ssum)/alpha ; sinv = alpha/|s|
                sl = slice(t0, t0 + n)
                snrm = small.tile([P, n], f32, tag="snrm", name=f"snrm{t0}")
                nc.scalar.activation(snrm, ssum_all[:, sl], AF.Sqrt,
                                     scale=float(1.0 / (ALPHA * ALPHA)))
                nc.vector.reciprocal(sinv_all[:, sl], snrm)

            def s_mul_transpose_evict(t, mul_on_dve, evict_on_dve):
                x = xs_s[t]
                s_n = work.tile([P, C], bf16, tag="s_n", name=f"sn{t}")
                if mul_on_dve:
                    nc.vector.tensor_scalar_mul(s_n, x, sinv_all[:, t:t + 1])
                else:
                    nc.scalar.mul(s_n, x, sinv_all[:, t:t + 1])
                psA = tpp.tile([P, KC * P], bf16, tag="tp", name=f"psA{t}")
                for c in range(KC):
                    nc.tensor.transpose(
                        psA[:, c * P:(c + 1) * P], s_n[:, c * P:(c + 1) * P],
                        ident)
                dst = s8T[:, 0:KC, t * P:(t + 1) * P]
                src = psA.rearrange("p (c n) -> p c n", c=KC)
                if evict_on_dve:
                    nc.vector.tensor_copy(dst, src)
                else:
                    nc.scalar.copy(dst, src)

            def q_sq(m):
                sq = work.tile([P, C], bf16, tag="sq")
                nc.scalar.activation(sq, qr_sb[:, m * C:(m + 1) * C],
                                     AF.Square,
                                     accum_out=qsum_all[:, m:m + 1])

            # ---- prologue prep: support tiles 0-3 ----
            # pairwise finish so the first transposes start after two squares
            s_dma_ahead(7)
            s_sq(0)
            s_sq(1)
            s_finish(0, 2)
            s_sq(2)
            s_mul_transpose_evict(0, mul_on_dve=True, evict_on_dve=False)
            s_sq(3)
            s_mul_transpose_evict(1, mul_on_dve=True, evict_on_dve=True)
            s_finish(2, 2)
            s_mul_transpose_evict(2, mul_on_dve=True, evict_on_dve=False)
            s_mul_transpose_evict(3, mul_on_dve=True, evict_on_dve=True)

            # per-pass schedule slots (pass w<4 preps tiles 4w+4..4w+7):
            # squares early, norm finish, prep units with slack before the
            # pass boundary, q squares interleaved
            SQ_SLOTS = (0, 1, 3, 5)
            UNIT_SLOTS = (7, 9, 11, 13)
            QSQ_SLOTS = (2, 4, 8, 10)

            mxs = [None] * MT
            prev = [None, None]
            for w in range(WAY):
                for m in range(MT):
                    if w < 4:
                        t_new = 4 * (w + 1)
                        if m == 0:
                            s_dma_ahead(t_new + 7)
                        if m in SQ_SLOTS:
                            s_sq(t_new + SQ_SLOTS.index(m))
                        if m == 6:
                            s_finish(t_new, 4)
                        if m in UNIT_SLOTS:
                            t = t_new + UNIT_SLOTS.index(m)
                            s_mul_transpose_evict(
                                t, mul_on_dve=(t % 4 == 0),
                                evict_on_dve=False)
                        if m in QSQ_SLOTS:
                            mq = 4 * w + QSQ_SLOTS.index(m)
                            if mq < MT:
                                q_sq(mq)
                    if w == 4 and m == 0:
                        # qinv = 1 / (k * alpha * |q_patch|)
                        kn = small.tile([P, MT], f32, tag="kn")
                        nc.scalar.activation(
                            kn, qsum_all, AF.Sqrt,
                            scale=float(k * k * ALPHA * ALPHA))
                        nc.vector.reciprocal(qinv, kn)
                    if w == 0:
                        mxs[m] = mxp.tile([P, WAY * 8], f32, tag="mx",
                                          name=f"mx{m}")
                    psc = spp.tile([P, NW], f32, tag="psc",
                                   name=f"psc{m}_{w}")
                    for j in range(KP):
                        nc.tensor.matmul(
                            psc,
                            qT_sb[:, 2 * j:2 * j + 2, m * P:(m + 1) * P],
                            s8T[:, 2 * j:2 * j + 2, w * NW:(w + 1) * NW],
                            start=(j == 0),
                            stop=(j == KP - 1),
                            perf_mode=DR,
                        )
                    nc.vector.max(mxs[m][:, w * 8:(w + 1) * 8], psc)
                    if w == WAY - 1:
                        tsum = small.tile([P, WAY], f32, tag="tsum")
                        nc.vector.tensor_reduce(
                            tsum,
                            mxs[m].rearrange("p (w j) -> p w j", w=WAY)[:, :, :k],
                            axis=mybir.AxisListType.X,
                            op=mybir.AluOpType.add,
                        )
                        sc_dt = bf16 if SCB else f32
                        # ACT is otherwise idle in the final pass
                        scaled = small.tile([P, WAY], sc_dt, tag="scaled")
                        nc.scalar.mul(scaled, tsum, qinv[:, m:m + 1])
                        if prev[0] is not None:
                            nc.tensor.matmul(
                                out_ps,
                                ind_sb[:, prev[1] * NQ:(prev[1] + 1) * NQ],
                                prev[0], start=(prev[1] == 0), stop=False)
                        prev = [scaled, m]
            nc.tensor.matmul(
                out_ps, ind_sb[:, prev[1] * NQ:(prev[1] + 1) * NQ],
                prev[0], start=False, stop=True)
            out_sb = small.tile([NQ, WAY], f32, tag="out_sb")
            nc.scalar.copy(out_sb, out_ps)
            nc.sync.dma_start(out=out_d, in_=out_sb)

    nc.compile()
    return nc


def get_program(k: int):
    if k not in _prog_cache:
        _prog_cache[k] = _build(k)
    return _prog_cache[k]


def make_in_maps(input1: np.ndarray, input2: np.ndarray):
    """Shard full inputs into per-core input maps (layout + dtype only)."""
    input1 = np.ascontiguousarray(np.asarray(input1), dtype=np.float32)
    input2 = np.ascontiguousarray(np.asarray(input2), dtype=np.float32)
    fp8 = ml_dtypes.float8_e4m3fn
    bf16 = ml_dtypes.bfloat16
    in_maps = []
    for core in range(N_CORES):
        b = core // 4
        qs = (core % 4) * NQ
        qe = min(Q, qs + NQ)
        nq = qe - qs
        qdat = input1[b].reshape(Q, HW, C)[qs:qe].reshape(-1, C)
        qfull = np.ones((PAD_P, C), np.float32)
        qfull[: nq * HW] = qdat
        # qT: fp8 banded transpose [128, 6, PAD_P] (band 5 zeros)
        q8 = qfull.astype(fp8)
        qT = np.zeros((P, 2 * KP, PAD_P), fp8)
        for c in range(KC):
            qT[:, c, :] = q8[:, c * P:(c + 1) * P].T
        # q row-major bf16, partition-major tiles [128, MT, C]
        qr = np.ascontiguousarray(
            qfull.reshape(MT, P, C).transpose(1, 0, 2)).astype(bf16)
        sfull = np.ones((PAD_S, C), np.float32)
        sfull[:NS] = input2[b].reshape(NS, C)
        sr = np.ascontiguousarray(
            sfull.reshape(ST, P, C).transpose(1, 0, 2)).astype(bf16)
        # indicator: patch row p of M-tile t belongs to query (t*128+p)//HW
        ind = np.zeros((P, MT * NQ), np.float32)
        g = np.arange(MT * P)
        j = g // HW
        valid = j < nq
        ind[g[valid] % P, (g[valid] // P) * NQ + j[valid]] = 1.0
        in_maps.append({
            "qT": qT.reshape(P, 2 * KP * PAD_P),
            "qr": qr.reshape(P, MT * C),
            "sr": sr.reshape(P, ST * C),
            "zs": np.zeros((P, PAD_S), fp8),
            "ind": ind.astype(bf16) if SCB else ind,
            "ident": np.eye(P).astype(bf16),
        })
    return in_maps


def gather_out(results) -> np.ndarray:
    out = np.zeros((B, Q, WAY), np.float32)
    for core in range(N_CORES):
        b = core // 4
        qs = (core % 4) * NQ
        n = min(Q, qs + NQ) - qs
        out[b, qs:qs + n] = results[core]["out"][:n]
    return out


def kernel(input1, input2, neighbor_k):
    k = int(np.asarray(neighbor_k))
    nc = get_program(k)
    in_maps = make_in_maps(input1, input2)
    # the axon-tunneled device occasionally reports a transient
    # "unrecoverable" state right after a previous process's teardown;
    # it recovers within seconds, so retry a couple of times
    import time
    last = None
    for attempt in range(3):
        try:
            res = run_bass_kernel_spmd(
                nc, in_maps, core_ids=list(range(N_CORES)))
            return gather_out(res.results)
        except Exception as e:  # noqa: BLE001
            last = e
            if attempt < 2:
                time.sleep(20.0 * (attempt + 1))
    raise last


# revision 23
# speedup vs baseline: 1.0335x; 1.0335x over previous
"""TRN2 Bass kernel for nn_MetaBaseline (DN4-style local-descriptor kNN).

Reference computation (per batch b):
  q = input1[b].reshape(7500, 640)           # query patch descriptors
  s = normalize(input2[b].reshape(2500, 640), axis=-1)
  scores = q @ s.T                           # [7500, 2500]
  per way group g (columns [500g, 500g+500)): top-k per row, mean,
  divide by |q_patch| (commutes with top-k since it is per-row), then
  sum over the 100 patches of each query -> out [75, 5].

Sharding: data-parallel over (b, query-quarter): 8 cores, each handles one
batch's quarter of queries (19 queries padded) with that batch's full
support replicated.

Implementation highlights vs the fp32r baseline:
- Score matmuls run in fp8e4m3 with MatmulPerfMode.DoubleRow (two 128-row
  k-tiles per pass): C=640 is padded to 6 chunks of 128 (band 5 zeros) so
  each (patch-tile, way) pair needs just 3 PE instructions.
- The query operand is uploaded pre-transposed in fp8 from the host
  (layout + dtype conversion only), so queries need no PE transpose, no
  PSUM eviction and no quantize pass on device. A second row-major bf16
  copy of q feeds the |q_patch| norm pass (ACT square+accum), matching the
  reference's exact-norm division.
- Support arrives row-major bf16; on device: ACT square+accum -> batched
  sqrt(scale=1/alpha^2) -> DVE reciprocal gives sinv = alpha/|s| per
  descriptor; DVE tensor_scalar (4x mode on bf16) or ACT scales to
  alpha-normalized bf16; PE transposes (bf16 identity, 1 cycle/row) into
  one PSUM bank per tile; a single strided copy evicts+converts to the
  fp8 banded layout. alpha=32 keeps e4m3 operands in the normal range;
  1/alpha folds into the per-query scale sqrt(k^2 alpha^2 sum q^2).
- Top-8 per (patch, way) via DVE max8 straight from the PSUM score bank;
  pass 4 finishes each patch tile with a strided top-k tensor_reduce, an
  ACT scale by 1/(k*alpha*|q_patch|), and a small fp32 indicator matmul
  accumulating per-query sums in PSUM -> [19, 5].
"""
import os
from contextlib import ExitStack

import ml_dtypes
import numpy as np

import concourse.bass as bass  # noqa: F401
import concourse.mybir as mybir
import concourse.tile as tile
from concourse import bacc
from concourse.bass_utils import run_bass_kernel_spmd

# Problem geometry (hardcoded per contest rules)
B, Q, WAY, SHOT, H, W, C = 2, 75, 5, 5, 10, 10, 640
HW = H * W               # 100 patches per query / support image
NQ = 19                  # queries per core (4 cores x 19 = 76 >= 75)
MT = 15                  # patch M-tiles of 128 -> 1920 rows (1900 real)
PAD_P = MT * 128
NS = WAY * SHOT * HW     # 2500 support descriptors per batch
ST = 20                  # support tiles of 128 -> 2560 rows
PAD_S = ST * 128
KC = 5                   # real C chunks of 128 (640 = 5*128)
KP = 3                   # DoubleRow chunk pairs (6 bands incl. zero band)
P = 128
NW = SHOT * HW           # 500 support descriptors per way group
N_CORES = 8
ALPHA = 32.0             # support operand scale for fp8 normal range
N_WARM = int(os.environ.get("N_WARM", "44"))
# NOTE: tensor_tensor_reduce with bf16 inputs faults on real TRN2 hardware
# (verified by bisection); squares therefore run on ACT only.
SQ_DVE = os.environ.get("SQ_DVE", "0") == "1"   # squares split ACT/DVE
TPP_BUFS = int(os.environ.get("TPP", "3"))       # transpose PSUM banks
SCB = os.environ.get("SCB", "1") == "1"          # scaled on DVE bf16 + bf16 ind

_prog_cache: dict[int, object] = {}


def _build(k: int):
    """Build + compile the per-core SPMD program for neighbor_k == k."""
    assert 1 <= k <= 8, f"neighbor_k={k} not supported (need 1..8)"
    nc = bacc.Bacc("TRN2", target_bir_lowering=False, debug=False)
    f32 = mybir.dt.float32
    bf16 = mybir.dt.bfloat16
    fp8 = mybir.dt.float8e4
    AF = mybir.ActivationFunctionType
    DR = mybir.MatmulPerfMode.DoubleRow

    qT_d = nc.dram_tensor("qT", [P, 2 * KP * PAD_P], fp8,
                          kind="ExternalInput").ap()
    qr_d = nc.dram_tensor("qr", [P, MT * C], bf16, kind="ExternalInput").ap()
    sr_d = nc.dram_tensor("sr", [P, ST * C], bf16, kind="ExternalInput").ap()
    zs_d = nc.dram_tensor("zs", [P, PAD_S], fp8, kind="ExternalInput").ap()
    ind_dt = bf16 if SCB else f32
    ind_d = nc.dram_tensor("ind", [P, MT * NQ], ind_dt,
                           kind="ExternalInput").ap()
    ident_d = nc.dram_tensor("ident", [P, P], bf16, kind="ExternalInput").ap()
    out_d = nc.dram_tensor("out", [NQ, WAY], f32, kind="ExternalOutput").ap()

    with tile.TileContext(nc) as tc:
        with ExitStack() as ctx:
            const = ctx.enter_context(tc.tile_pool(name="const", bufs=1))
            big = ctx.enter_context(tc.tile_pool(name="big", bufs=1))
            loads = ctx.enter_context(tc.tile_pool(name="loads", bufs=8))
            work = ctx.enter_context(tc.tile_pool(name="work", bufs=3))
            small = ctx.enter_context(tc.tile_pool(name="small", bufs=4))
            outp = ctx.enter_context(
                tc.tile_pool(name="outp", bufs=1, space="PSUM")
            )
            tpp = ctx.enter_context(
                tc.tile_pool(name="tpp", bufs=TPP_BUFS, space="PSUM")
            )
            warmp = ctx.enter_context(
                tc.tile_pool(name="warmp", bufs=1, space="PSUM")
            )
            spp = ctx.enter_context(
                tc.tile_pool(name="spp", bufs=3, space="PSUM")
            )

            ident = const.tile([P, P], bf16)
            ind_sb = const.tile([P, MT * NQ], ind_dt)
            # banded transposed operands: [partition, chunk band, column]
            s8T = big.tile([P, 2 * KP, PAD_S], fp8, name="s8T")
            qT_sb = big.tile([P, 2 * KP, PAD_P], fp8, name="qT_sb")
            qr_sb = big.tile([P, MT * C], bf16, name="qr_sb")

            # batched norm scalars
            ssum_all = const.tile([P, ST], f32, name="ssum")
            sinv_all = const.tile([P, ST], f32, name="sinv")
            qsum_all = const.tile([P, MT], f32, name="qsum")
            qinv = const.tile([P, MT], f32, name="qinv")

            out_ps = outp.tile([NQ, WAY], f32)

            # ---- warmups: ACT tables + PE pipeline + pstate ramp ----
            wtile = const.tile([P, P], bf16, name="wtile")
            nc.vector.memset(wtile, 1.0)
            wsq = small.tile([P, P], bf16, tag="wsq")
            wss = small.tile([P, 1], f32, tag="snrm")
            nc.scalar.activation(wsq, wtile, AF.Square, accum_out=wss)
            nc.scalar.sqrt(wss, wss)
            wps = warmp.tile([P, 4 * P], f32, tag="warm")
            for i in range(N_WARM):
                nc.tensor.matmul(
                    wps[:, (i % 4) * P:(i % 4 + 1) * P], wtile, wtile,
                    start=True, stop=True)

            # ---- prologue DMAs ----
            xs_s = [None] * ST

            def s_dma(t):
                x = loads.tile([P, C], bf16, tag="x_tile", name=f"sx{t}")
                nc.sync.dma_start(out=x, in_=sr_d[:, t * C:(t + 1) * C])
                xs_s[t] = x

            for t in range(4):
                s_dma(t)
            nc.sync.dma_start(out=ident, in_=ident_d)
            nc.sync.dma_start(out=qT_sb, in_=qT_d)
            nc.sync.dma_start(out=s8T[:, 2 * KP - 1, :], in_=zs_d)
            nc.sync.dma_start(out=ind_sb, in_=ind_d)
            nc.sync.dma_start(out=qr_sb, in_=qr_d)

            next_s = [4]

            def s_dma_ahead(upto):
                while next_s[0] <= min(upto, ST - 1):
                    s_dma(next_s[0])
                    next_s[0] += 1

            # ---- prep helpers ----
            def s_sq(t, on_dve=False):
                if on_dve and SQ_DVE:
                    sqf = work.tile([P, C], f32, tag="sqf")
                    nc.vector.tensor_tensor_reduce(
                        sqf, xs_s[t], xs_s[t], 1.0, 0.0,
                        mybir.AluOpType.mult, mybir.AluOpType.add,
                        ssum_all[:, t:t + 1])
                else:
                    sq = work.tile([P, C], bf16, tag="sq")
                    nc.scalar.activation(sq, xs_s[t], AF.Square,
                                         accum_out=ssum_all[:, t:t + 1])

            def s_finish(t0, n):
                # tiles t0..t0+n-1: snrm = sqrt(ssum)/alpha ; sinv = alpha/|s|
                sl = slice(t0, t0 + n)
                snrm = small.tile([P, n], f32, tag="snrm", name=f"snrm{t0}")
                nc.scalar.activation(snrm, ssum_all[:, sl], AF.Sqrt,
                                     scale=float(1.0 / (ALPHA * ALPHA)))
                nc.vector.reciprocal(sinv_all[:, sl], snrm)

            def s_mul_transpose_evict(t, mul_on_dve, evict_on_dve):
                x = xs_s[t]
                s_n = work.tile([P, C], bf16, tag="s_n", name=f"sn{t}")
                if mul_on_dve:
                    nc.vector.tensor_scalar_mul(s_n, x, sinv_all[:, t:t + 1])
                else:
                    nc.scalar.mul(s_n, x, sinv_all[:, t:t + 1])
                psA = tpp.tile([P, KC * P], bf16, tag="tp", name=f"psA{t}")
                for c in range(KC):
                    nc.tensor.transpose(
                        psA[:, c * P:(c + 1) * P], s_n[:, c * P:(c + 1) * P],
                        ident)
                dst = s8T[:, 0:KC, t * P:(t + 1) * P]
                src = psA.rearrange("p (c n) -> p c n", c=KC)
                if evict_on_dve:
                    nc.vector.tensor_copy(dst, src)
                else:
                    nc.scalar.copy(dst, src)

            def q_sq(m):
                sq = work.tile([P, C], bf16, tag="sq")
                nc.scalar.activation(sq, qr_sb[:, m * C:(m + 1) * C],
                                     AF.Square,
                                     accum_out=qsum_all[:, m:m + 1])

            # ---- prologue prep: support tiles 0-3 ----
            # pairwise finish so the first transposes start after two squares
            s_dma_ahead(7)
            s_sq(0)
            s_sq(1)
            s_finish(0, 2)
            s_sq(2)
            s_sq(3)
            s_mul_transpose_evict(0, mul_on_dve=True, evict_on_dve=False)
            s_finish(2, 2)
            s_mul_transpose_evict(1, mul_on_dve=True, evict_on_dve=True)
            s_mul_transpose_evict(2, mul_on_dve=True, evict_on_dve=False)
            s_mul_transpose_evict(3, mul_on_dve=True, evict_on_dve=True)

            # per-pass schedule slots (pass w<4 preps tiles 4w+4..4w+7):
            # squares early, norm finish, prep units with slack before the
            # pass boundary, q squares interleaved
            SQ_SLOTS = (0, 1, 2, 3)
            UNIT_SLOTS = (5, 7, 9, 11)
            QSQ_SLOTS = (6, 8, 10, 12)

            # all top-8 results live in one tile so pass-4 can reduce them
            # in fused 5-tile groups (one DVE tensor_reduce per group)
            mxs_all = const.tile([P, MT * WAY * 8], f32, name="mxs_all")
            tsum_all = const.tile([P, MT * WAY], f32, name="tsum_all")
            mx4 = mxs_all.rearrange("p (m w j) -> p m w j", m=MT, w=WAY)
            sc_dt = bf16 if SCB else f32
            n_emit = [0]
            prev = [None, None]
            pend = []

            def scaled_ind(m):
                # ACT is otherwise idle in the final pass
                scaled = small.tile([P, WAY], sc_dt, tag="scaled")
                nc.scalar.mul(scaled, tsum_all[:, m * WAY:(m + 1) * WAY],
                              qinv[:, m:m + 1])
                if prev[0] is not None:
                    nc.tensor.matmul(
                        out_ps,
                        ind_sb[:, prev[1] * NQ:(prev[1] + 1) * NQ],
                        prev[0], start=(n_emit[0] == 0), stop=False)
                    n_emit[0] += 1
                prev[0], prev[1] = scaled, m

            for w in range(WAY):
                for m in range(MT):
                    if w < 4:
                        t_new = 4 * (w + 1)
                        if m == 0:
                            s_dma_ahead(t_new + 7)
                        if m in SQ_SLOTS:
                            s_sq(t_new + SQ_SLOTS.index(m))
                        if m == 4:
                            s_finish(t_new, 4)
                        if m in UNIT_SLOTS:
                            t = t_new + UNIT_SLOTS.index(m)
                            s_mul_transpose_evict(
                                t, mul_on_dve=(t % 4 == 3),
                                evict_on_dve=(t % 4 == 1))
                        if m in QSQ_SLOTS:
                            mq = 4 * w + QSQ_SLOTS.index(m)
                            if mq < MT:
                                q_sq(mq)
                    if w == 4 and m == 0:
                        # qinv = 1 / (k * alpha * |q_patch|)
                        kn = small.tile([P, MT], f32, tag="kn")
                        nc.scalar.activation(
                            kn, qsum_all, AF.Sqrt,
                            scale=float(k * k * ALPHA * ALPHA))
                        nc.vector.reciprocal(qinv, kn)
                    psc = spp.tile([P, NW], f32, tag="psc",
                                   name=f"psc{m}_{w}")
                    for j in range(KP):
                        nc.tensor.matmul(
                            psc,
                            qT_sb[:, 2 * j:2 * j + 2, m * P:(m + 1) * P],
                            s8T[:, 2 * j:2 * j + 2, w * NW:(w + 1) * NW],
                            start=(j == 0),
                            stop=(j == KP - 1),
                            perf_mode=DR,
                        )
                    nc.vector.max(
                        mxs_all[:, (m * WAY + w) * 8:(m * WAY + w + 1) * 8],
                        psc)
                    if w == WAY - 1:
                        if m in (4, 9, 14):
                            g0 = m - 4
                            nc.vector.tensor_reduce(
                                tsum_all[:, g0 * WAY:(m + 1) * WAY],
                                mx4[:, g0:m + 1, :, :k],
                                axis=mybir.AxisListType.X,
                                op=mybir.AluOpType.add,
                            )
                            pend.extend(range(g0, m + 1))
                        if pend:
                            scaled_ind(pend.pop(0))
            while pend:
                scaled_ind(pend.pop(0))
            nc.tensor.matmul(
                out_ps, ind_sb[:, prev[1] * NQ:(prev[1] + 1) * NQ],
                prev[0], start=False, stop=True)
            out_sb = small.tile([NQ, WAY], f32, tag="out_sb")
            nc.vector.tensor_copy(out_sb, out_ps)
            nc.sync.dma_start(out=out_d, in_=out_sb)

    nc.compile()
    return nc


def get_program(k: int):
    if k not in _prog_cache:
        _prog_cache[k] = _build(k)
    return _prog_cache[k]


def make_in_maps(input1: np.ndarray, input2: np.ndarray):
    """Shard full inputs into per-core input maps (layout + dtype only)."""
    input1 = np.ascontiguousarray(np.asarray(input1), dtype=np.float32)
    input2 = np.ascontiguousarray(np.asarray(input2), dtype=np.float32)
    fp8 = ml_dtypes.float8_e4m3fn
    bf16 = ml_dtypes.bfloat16
    in_maps = []
    for core in range(N_CORES):
        b = core // 4
        qs = (core % 4) * NQ
        qe = min(Q, qs + NQ)
        nq = qe - qs
        qdat = input1[b].reshape(Q, HW, C)[qs:qe].reshape(-1, C)
        qfull = np.ones((PAD_P, C), np.float32)
        qfull[: nq * HW] = qdat
        # qT: fp8 banded transpose [128, 6, PAD_P] (band 5 zeros)
        q8 = qfull.astype(fp8)
        qT = np.zeros((P, 2 * KP, PAD_P), fp8)
        for c in range(KC):
            qT[:, c, :] = q8[:, c * P:(c + 1) * P].T
        # q row-major bf16, partition-major tiles [128, MT, C]
        qr = np.ascontiguousarray(
            qfull.reshape(MT, P, C).transpose(1, 0, 2)).astype(bf16)
        sfull = np.ones((PAD_S, C), np.float32)
        sfull[:NS] = input2[b].reshape(NS, C)
        sr = np.ascontiguousarray(
            sfull.reshape(ST, P, C).transpose(1, 0, 2)).astype(bf16)
        # indicator: patch row p of M-tile t belongs to query (t*128+p)//HW
        ind = np.zeros((P, MT * NQ), np.float32)
        g = np.arange(MT * P)
        j = g // HW
        valid = j < nq
        ind[g[valid] % P, (g[valid] // P) * NQ + j[valid]] = 1.0
        in_maps.append({
            "qT": qT.reshape(P, 2 * KP * PAD_P),
            "qr": qr.reshape(P, MT * C),
            "sr": sr.reshape(P, ST * C),
            "zs": np.zeros((P, PAD_S), fp8),
            "ind": ind.astype(bf16) if SCB else ind,
            "ident": np.eye(P).astype(bf16),
        })
    return in_maps


def gather_out(results) -> np.ndarray:
    out = np.zeros((B, Q, WAY), np.float32)
    for core in range(N_CORES):
        b = core // 4
        qs = (core % 4) * NQ
        n = min(Q, qs + NQ) - qs
        out[b, qs:qs + n] = results[core]["out"][:n]
    return out


def kernel(input1, input2, neighbor_k):
    k = int(np.asarray(neighbor_k))
    nc = get_program(k)
    in_maps = make_in_maps(input1, input2)
    # the axon-tunneled device occasionally reports a transient
    # "unrecoverable" state right after a previous process's teardown;
    # it recovers within seconds, so retry a couple of times
    import time
    last = None
    for attempt in range(3):
        try:
            res = run_bass_kernel_spmd(
                nc, in_maps, core_ids=list(range(N_CORES)))
            return gather_out(res.results)
        except Exception as e:  # noqa: BLE001
            last = e
            if attempt < 2:
                time.sleep(20.0 * (attempt + 1))
    raise last


# revision 25
# speedup vs baseline: 1.0491x; 1.0151x over previous
"""TRN2 Bass kernel for nn_MetaBaseline (DN4-style local-descriptor kNN).

Reference computation (per batch b):
  q = input1[b].reshape(7500, 640)           # query patch descriptors
  s = normalize(input2[b].reshape(2500, 640), axis=-1)
  scores = q @ s.T                           # [7500, 2500]
  per way group g (columns [500g, 500g+500)): top-k per row, mean,
  divide by |q_patch| (commutes with top-k since it is per-row), then
  sum over the 100 patches of each query -> out [75, 5].

Sharding: data-parallel over (b, query-quarter): 8 cores, each handles one
batch's quarter of queries (19 queries padded) with that batch's full
support replicated.

Implementation highlights vs the fp32r baseline:
- Score matmuls run in fp8e4m3 with MatmulPerfMode.DoubleRow (two 128-row
  k-tiles per pass): C=640 is padded to 6 chunks of 128 (band 5 zeros) so
  each (patch-tile, way) pair needs just 3 PE instructions.
- The query operand is uploaded pre-transposed in fp8 from the host
  (layout + dtype conversion only), so queries need no PE transpose, no
  PSUM eviction and no quantize pass on device. A second row-major bf16
  copy of q feeds the |q_patch| norm pass (ACT square+accum), matching the
  reference's exact-norm division.
- Support arrives row-major bf16; on device: ACT square+accum -> batched
  sqrt(scale=1/alpha^2) -> DVE reciprocal gives sinv = alpha/|s| per
  descriptor; DVE tensor_scalar (4x mode on bf16) or ACT scales to
  alpha-normalized bf16; PE transposes (bf16 identity, 1 cycle/row) into
  one PSUM bank per tile; a single strided copy evicts+converts to the
  fp8 banded layout. alpha=32 keeps e4m3 operands in the normal range;
  1/alpha folds into the per-query scale sqrt(k^2 alpha^2 sum q^2).
- Top-8 per (patch, way) via DVE max8 straight from the PSUM score bank;
  pass 4 finishes each patch tile with a strided top-k tensor_reduce, an
  ACT scale by 1/(k*alpha*|q_patch|), and a small fp32 indicator matmul
  accumulating per-query sums in PSUM -> [19, 5].
"""
import os
from contextlib import ExitStack

import ml_dtypes
import numpy as np

import concourse.bass as bass  # noqa: F401
import concourse.mybir as mybir
import concourse.tile as tile
from concourse import bacc
from concourse.bass_utils import run_bass_kernel_spmd

# Problem geometry (hardcoded per contest rules)
B, Q, WAY, SHOT, H, W, C = 2, 75, 5, 5, 10, 10, 640
HW = H * W               # 100 patches per query / support image
NQ = 19                  # queries per core (4 cores x 19 = 76 >= 75)
MT = 15                  # patch M-tiles of 128 -> 1920 rows (1900 real)
PAD_P = MT * 128
NS = WAY * SHOT * HW     # 2500 support descriptors per batch
ST = 20                  # support tiles of 128 -> 2560 rows
PAD_S = ST * 128
KC = 5                   # real C chunks of 128 (640 = 5*128)
KP = 3                   # DoubleRow chunk pairs (6 bands incl. zero band)
P = 128
NW = SHOT * HW           # 500 support descriptors per way group
N_CORES = 8
ALPHA = 32.0             # support operand scale for fp8 normal range
N_WARM = int(os.environ.get("N_WARM", "32"))
# NOTE: tensor_tensor_reduce with bf16 inputs faults on real TRN2 hardware
# (verified by bisection); squares therefore run on ACT only.
SQ_DVE = os.environ.get("SQ_DVE", "0") == "1"   # squares split ACT/DVE
TPP_BUFS = int(os.environ.get("TPP", "3"))       # transpose PSUM banks
SCB = os.environ.get("SCB", "1") == "1"          # scaled on DVE bf16 + bf16 ind

_prog_cache: dict[int, object] = {}


def _build(k: int):
    """Build + compile the per-core SPMD program for neighbor_k == k."""
    assert 1 <= k <= 8, f"neighbor_k={k} not supported (need 1..8)"
    nc = bacc.Bacc("TRN2", target_bir_lowering=False, debug=False)
    f32 = mybir.dt.float32
    bf16 = mybir.dt.bfloat16
    fp8 = mybir.dt.float8e4
    AF = mybir.ActivationFunctionType
    DR = mybir.MatmulPerfMode.DoubleRow

    qT_d = nc.dram_tensor("qT", [P, 2 * KP * PAD_P], fp8,
                          kind="ExternalInput").ap()
    qr_d = nc.dram_tensor("qr", [P, MT * C], bf16, kind="ExternalInput").ap()
    sr_d = nc.dram_tensor("sr", [P, ST * C], bf16, kind="ExternalInput").ap()
    zs_d = nc.dram_tensor("zs", [P, PAD_S], fp8, kind="ExternalInput").ap()
    ind_dt = bf16 if SCB else f32
    ind_d = nc.dram_tensor("ind", [P, MT * NQ], ind_dt,
                           kind="ExternalInput").ap()
    ident_d = nc.dram_tensor("ident", [P, P], bf16, kind="ExternalInput").ap()
    out_d = nc.dram_tensor("out", [NQ, WAY], f32, kind="ExternalOutput").ap()

    with tile.TileContext(nc) as tc:
        with ExitStack() as ctx:
            const = ctx.enter_context(tc.tile_pool(name="const", bufs=1))
            big = ctx.enter_context(tc.tile_pool(name="big", bufs=1))
            loads = ctx.enter_context(tc.tile_pool(name="loads", bufs=8))
            work = ctx.enter_context(tc.tile_pool(name="work", bufs=3))
            small = ctx.enter_context(tc.tile_pool(name="small", bufs=4))
            mxp = ctx.enter_context(tc.tile_pool(name="mxp", bufs=MT))
            outp = ctx.enter_context(
                tc.tile_pool(name="outp", bufs=1, space="PSUM")
            )
            tpp = ctx.enter_context(
                tc.tile_pool(name="tpp", bufs=TPP_BUFS, space="PSUM")
            )
            warmp = ctx.enter_context(
                tc.tile_pool(name="warmp", bufs=1, space="PSUM")
            )
            spp = ctx.enter_context(
                tc.tile_pool(name="spp", bufs=3, space="PSUM")
            )

            ident = const.tile([P, P], bf16)
            ind_sb = const.tile([P, MT * NQ], ind_dt)
            # banded transposed operands: [partition, chunk band, column]
            s8T = big.tile([P, 2 * KP, PAD_S], fp8, name="s8T")
            qT_sb = big.tile([P, 2 * KP, PAD_P], fp8, name="qT_sb")
            qr_sb = big.tile([P, MT * C], bf16, name="qr_sb")

            # batched norm scalars
            ssum_all = const.tile([P, ST], f32, name="ssum")
            sinv_all = const.tile([P, ST], f32, name="sinv")
            qsum_all = const.tile([P, MT], f32, name="qsum")
            qinv = const.tile([P, MT], f32, name="qinv")

            out_ps = outp.tile([NQ, WAY], f32)

            # ---- warmups: ACT tables + PE pipeline + pstate ramp ----
            wtile = const.tile([P, P], bf16, name="wtile")
            nc.vector.memset(wtile, 1.0)
            wsq = small.tile([P, P], bf16, tag="wsq")
            wss = small.tile([P, 1], f32, tag="snrm")
            nc.scalar.activation(wsq, wtile, AF.Square, accum_out=wss)
            nc.scalar.sqrt(wss, wss)
            wps = warmp.tile([P, 4 * P], f32, tag="warm")
            for i in range(N_WARM):
                nc.tensor.matmul(
                    wps[:, (i % 4) * P:(i % 4 + 1) * P], wtile, wtile,
                    start=True, stop=True)

            # ---- prologue DMAs ----
            xs_s = [None] * ST

            def s_dma(t):
                x = loads.tile([P, C], bf16, tag="x_tile", name=f"sx{t}")
                nc.sync.dma_start(out=x, in_=sr_d[:, t * C:(t + 1) * C])
                xs_s[t] = x

            for t in range(4):
                s_dma(t)
            nc.sync.dma_start(out=ident, in_=ident_d)
            nc.sync.dma_start(out=qT_sb, in_=qT_d)
            nc.sync.dma_start(out=s8T[:, 2 * KP - 1, :], in_=zs_d)
            nc.sync.dma_start(out=ind_sb, in_=ind_d)

            next_s = [4]

            def s_dma_ahead(upto):
                while next_s[0] <= min(upto, ST - 1):
                    s_dma(next_s[0])
                    next_s[0] += 1

            # ---- prep helpers ----
            def s_sq(t, on_dve=False):
                if on_dve and SQ_DVE:
                    sqf = work.tile([P, C], f32, tag="sqf")
                    nc.vector.tensor_tensor_reduce(
                        sqf, xs_s[t], xs_s[t], 1.0, 0.0,
                        mybir.AluOpType.mult, mybir.AluOpType.add,
                        ssum_all[:, t:t + 1])
                else:
                    sq = work.tile([P, C], bf16, tag="sq")
                    nc.scalar.activation(sq, xs_s[t], AF.Square,
                                         accum_out=ssum_all[:, t:t + 1])

            def s_finish(t0, n):
                # tiles t0..t0+n-1: snrm = sqrt(ssum)/alpha ; sinv = alpha/|s|
                sl = slice(t0, t0 + n)
                snrm = small.tile([P, n], f32, tag="snrm", name=f"snrm{t0}")
                nc.scalar.activation(snrm, ssum_all[:, sl], AF.Sqrt,
                                     scale=float(1.0 / (ALPHA * ALPHA)))
                nc.vector.reciprocal(sinv_all[:, sl], snrm)

            def s_mul_transpose_evict(t, mul_on_dve, evict_on_dve):
                x = xs_s[t]
                s_n = work.tile([P, C], bf16, tag="s_n", name=f"sn{t}")
                if mul_on_dve:
                    nc.vector.tensor_scalar_mul(s_n, x, sinv_all[:, t:t + 1])
                else:
                    nc.scalar.mul(s_n, x, sinv_all[:, t:t + 1])
                psA = tpp.tile([P, KC * P], bf16, tag="tp", name=f"psA{t}")
                for c in range(KC):
                    nc.tensor.transpose(
                        psA[:, c * P:(c + 1) * P], s_n[:, c * P:(c + 1) * P],
                        ident)
                dst = s8T[:, 0:KC, t * P:(t + 1) * P]
                src = psA.rearrange("p (c n) -> p c n", c=KC)
                if evict_on_dve:
                    nc.vector.tensor_copy(dst, src)
                else:
                    nc.scalar.copy(dst, src)

            def q_sq(m):
                sq = work.tile([P, C], bf16, tag="sq")
                nc.scalar.activation(sq, qr_sb[:, m * C:(m + 1) * C],
                                     AF.Square,
                                     accum_out=qsum_all[:, m:m + 1])

            # ---- prologue prep: support tiles 0-3 ----
            # pairwise finish so the first transposes start after two squares
            s_dma_ahead(7)
            s_sq(0)
            s_sq(1)
            s_finish(0, 2)
            s_sq(2)
            s_sq(3)
            s_finish(2, 2)
            s_mul_transpose_evict(0, mul_on_dve=True, evict_on_dve=False)
            s_mul_transpose_evict(1, mul_on_dve=True, evict_on_dve=True)
            s_mul_transpose_evict(2, mul_on_dve=True, evict_on_dve=False)
            s_mul_transpose_evict(3, mul_on_dve=True, evict_on_dve=True)
            # query row-major copy is only needed from pass-0 q squares on;
            # issuing its large DMA here keeps prologue HBM bandwidth for
            # the critical support tiles
            nc.sync.dma_start(out=qr_sb, in_=qr_d)

            # per-pass schedule slots (pass w<4 preps tiles 4w+4..4w+7):
            # squares early, norm finish, prep units with slack before the
            # pass boundary, q squares interleaved
            SQ_SLOTS = (0, 1, 2, 3)
            UNIT_SLOTS = (5, 7, 9, 11)
            QSQ_SLOTS = (6, 8, 10, 12)

            mxs = [None] * MT
            sc_dt = bf16 if SCB else f32
            prev = [None, None]

            for w in range(WAY):
                for m in range(MT):
                    if w < 4:
                        t_new = 4 * (w + 1)
                        if m == 0:
                            s_dma_ahead(t_new + 7)
                        if m in SQ_SLOTS:
                            s_sq(t_new + SQ_SLOTS.index(m))
                        if m == 4:
                            s_finish(t_new, 4)
                        if m in UNIT_SLOTS:
                            t = t_new + UNIT_SLOTS.index(m)
                            s_mul_transpose_evict(
                                t, mul_on_dve=(t % 4 == 3),
                                evict_on_dve=(t % 4 == 1))
                        if m in QSQ_SLOTS:
                            mq = 4 * w + QSQ_SLOTS.index(m)
                            if mq < MT:
                                q_sq(mq)
                    if w == 4 and m == 0:
                        # qinv = 1 / (k * alpha * |q_patch|)
                        kn = small.tile([P, MT], f32, tag="kn")
                        nc.scalar.activation(
                            kn, qsum_all, AF.Sqrt,
                            scale=float(k * k * ALPHA * ALPHA))
                        nc.vector.reciprocal(qinv, kn)
                    if w == 0:
                        mxs[m] = mxp.tile([P, WAY * 8], f32, tag="mx",
                                          name=f"mx{m}")
                    psc = spp.tile([P, NW], f32, tag="psc",
                                   name=f"psc{m}_{w}")
                    for j in range(KP):
                        nc.tensor.matmul(
                            psc,
                            qT_sb[:, 2 * j:2 * j + 2, m * P:(m + 1) * P],
                            s8T[:, 2 * j:2 * j + 2, w * NW:(w + 1) * NW],
                            start=(j == 0),
                            stop=(j == KP - 1),
                            perf_mode=DR,
                        )
                    nc.vector.max(mxs[m][:, w * 8:(w + 1) * 8], psc)
                    if w == WAY - 1:
                        tsum = small.tile([P, WAY], f32, tag="tsum")
                        nc.vector.tensor_reduce(
                            tsum,
                            mxs[m].rearrange("p (w j) -> p w j", w=WAY)[:, :, :k],
                            axis=mybir.AxisListType.X,
                            op=mybir.AluOpType.add,
                        )
                        # ACT is otherwise idle in the final pass
                        scaled = small.tile([P, WAY], sc_dt, tag="scaled")
                        nc.scalar.mul(scaled, tsum, qinv[:, m:m + 1])
                        if prev[0] is not None:
                            nc.tensor.matmul(
                                out_ps,
                                ind_sb[:, prev[1] * NQ:(prev[1] + 1) * NQ],
                                prev[0], start=(prev[1] == 0), stop=False)
                        prev[0], prev[1] = scaled, m
            nc.tensor.matmul(
                out_ps, ind_sb[:, prev[1] * NQ:(prev[1] + 1) * NQ],
                prev[0], start=False, stop=True)
            out_sb = small.tile([NQ, WAY], f32, tag="out_sb")
            nc.vector.tensor_copy(out_sb, out_ps)
            nc.sync.dma_start(out=out_d, in_=out_sb)

    nc.compile()
    return nc


def get_program(k: int):
    if k not in _prog_cache:
        _prog_cache[k] = _build(k)
    return _prog_cache[k]


def make_in_maps(input1: np.ndarray, input2: np.ndarray):
    """Shard full inputs into per-core input maps (layout + dtype only)."""
    input1 = np.ascontiguousarray(np.asarray(input1), dtype=np.float32)
    input2 = np.ascontiguousarray(np.asarray(input2), dtype=np.float32)
    fp8 = ml_dtypes.float8_e4m3fn
    bf16 = ml_dtypes.bfloat16
    in_maps = []
    for core in range(N_CORES):
        b = core // 4
        qs = (core % 4) * NQ
        qe = min(Q, qs + NQ)
        nq = qe - qs
        qdat = input1[b].reshape(Q, HW, C)[qs:qe].reshape(-1, C)
        qfull = np.ones((PAD_P, C), np.float32)
        qfull[: nq * HW] = qdat
        # qT: fp8 banded transpose [128, 6, PAD_P] (band 5 zeros)
        q8 = qfull.astype(fp8)
        qT = np.zeros((P, 2 * KP, PAD_P), fp8)
        for c in range(KC):
            qT[:, c, :] = q8[:, c * P:(c + 1) * P].T
        # q row-major bf16, partition-major tiles [128, MT, C]
        qr = np.ascontiguousarray(
            qfull.reshape(MT, P, C).transpose(1, 0, 2)).astype(bf16)
        sfull = np.ones((PAD_S, C), np.float32)
        sfull[:NS] = input2[b].reshape(NS, C)
        sr = np.ascontiguousarray(
            sfull.reshape(ST, P, C).transpose(1, 0, 2)).astype(bf16)
        # indicator: patch row p of M-tile t belongs to query (t*128+p)//HW
        ind = np.zeros((P, MT * NQ), np.float32)
        g = np.arange(MT * P)
        j = g // HW
        valid = j < nq
        ind[g[valid] % P, (g[valid] // P) * NQ + j[valid]] = 1.0
        in_maps.append({
            "qT": qT.reshape(P, 2 * KP * PAD_P),
            "qr": qr.reshape(P, MT * C),
            "sr": sr.reshape(P, ST * C),
            "zs": np.zeros((P, PAD_S), fp8),
            "ind": ind.astype(bf16) if SCB else ind,
            "ident": np.eye(P).astype(bf16),
        })
    return in_maps


def gather_out(results) -> np.ndarray:
    out = np.zeros((B, Q, WAY), np.float32)
    for core in range(N_CORES):
        b = core // 4
        qs = (core % 4) * NQ
        n = min(Q, qs + NQ) - qs
        out[b, qs:qs + n] = results[core]["out"][:n]
    return out


def kernel(input1, input2, neighbor_k):
    k = int(np.asarray(neighbor_k))
    nc = get_program(k)
    in_maps = make_in_maps(input1, input2)
    # the axon-tunneled device occasionally reports a transient
    # "unrecoverable" state right after a previous process's teardown;
    # it recovers within seconds, so retry a couple of times
    import time
    last = None
    for attempt in range(3):
        try:
            res = run_bass_kernel_spmd(
                nc, in_maps, core_ids=list(range(N_CORES)))
            return gather_out(res.results)
        except Exception as e:  # noqa: BLE001
            last = e
            if attempt < 2:
                time.sleep(20.0 * (attempt + 1))
    raise last


# revision 26
# speedup vs baseline: 1.0875x; 1.0366x over previous
"""TRN2 Bass kernel for nn_MetaBaseline (DN4-style local-descriptor kNN).

Reference computation (per batch b):
  q = input1[b].reshape(7500, 640)           # query patch descriptors
  s = normalize(input2[b].reshape(2500, 640), axis=-1)
  scores = q @ s.T                           # [7500, 2500]
  per way group g (columns [500g, 500g+500)): top-k per row, mean,
  divide by |q_patch| (commutes with top-k since it is per-row), then
  sum over the 100 patches of each query -> out [75, 5].

Sharding: data-parallel over (b, query-quarter): 8 cores, each handles one
batch's quarter of queries (19 queries padded) with that batch's full
support replicated.

Implementation highlights vs the fp32r baseline:
- Score matmuls run in fp8e4m3 with MatmulPerfMode.DoubleRow (two 128-row
  k-tiles per pass): C=640 is padded to 6 chunks of 128 (band 5 zeros) so
  each (patch-tile, way) pair needs just 3 PE instructions.
- The query operand is uploaded pre-transposed in fp8 from the host
  (layout + dtype conversion only), so queries need no PE transpose, no
  PSUM eviction and no quantize pass on device. A second row-major bf16
  copy of q feeds the |q_patch| norm pass (ACT square+accum), matching the
  reference's exact-norm division.
- Support arrives row-major bf16; on device: ACT square+accum -> batched
  sqrt(scale=1/alpha^2) -> DVE reciprocal gives sinv = alpha/|s| per
  descriptor; DVE tensor_scalar (4x mode on bf16) or ACT scales to
  alpha-normalized bf16; PE transposes (bf16 identity, 1 cycle/row) into
  one PSUM bank per tile; a single strided copy evicts+converts to the
  fp8 banded layout. alpha=32 keeps e4m3 operands in the normal range;
  1/alpha folds into the per-query scale sqrt(k^2 alpha^2 sum q^2).
- Top-8 per (patch, way) via DVE max8 straight from the PSUM score bank;
  pass 4 finishes each patch tile with a strided top-k tensor_reduce, an
  ACT scale by 1/(k*alpha*|q_patch|), and a small fp32 indicator matmul
  accumulating per-query sums in PSUM -> [19, 5].
"""
import os
from contextlib import ExitStack

import ml_dtypes
import numpy as np

import concourse.bass as bass  # noqa: F401
import concourse.mybir as mybir
import concourse.tile as tile
from concourse import bacc
from concourse.bass_utils import run_bass_kernel_spmd

# Problem geometry (hardcoded per contest rules)
B, Q, WAY, SHOT, H, W, C = 2, 75, 5, 5, 10, 10, 640
HW = H * W               # 100 patches per query / support image
NQ = 19                  # queries per core (4 cores x 19 = 76 >= 75)
MT = 15                  # patch M-tiles of 128 -> 1920 rows (1900 real)
PAD_P = MT * 128
NS = WAY * SHOT * HW     # 2500 support descriptors per batch
ST = 20                  # support tiles of 128 -> 2560 rows
PAD_S = ST * 128
KC = 5                   # real C chunks of 128 (640 = 5*128)
KP = 3                   # DoubleRow chunk pairs (6 bands incl. zero band)
P = 128
NW = SHOT * HW           # 500 support descriptors per way group
N_CORES = 8
ALPHA = 32.0             # support operand scale for fp8 normal range
N_WARM = int(os.environ.get("N_WARM", "36"))
# NOTE: tensor_tensor_reduce with bf16 inputs faults on real TRN2 hardware
# (verified by bisection); squares therefore run on ACT only.
SQ_DVE = os.environ.get("SQ_DVE", "0") == "1"   # squares split ACT/DVE
TPP_BUFS = int(os.environ.get("TPP", "3"))       # transpose PSUM banks
SCB = os.environ.get("SCB", "1") == "1"          # scaled on DVE bf16 + bf16 ind

_prog_cache: dict[int, object] = {}


def _build(k: int):
    """Build + compile the per-core SPMD program for neighbor_k == k."""
    assert 1 <= k <= 8, f"neighbor_k={k} not supported (need 1..8)"
    nc = bacc.Bacc("TRN2", target_bir_lowering=False, debug=False)
    f32 = mybir.dt.float32
    bf16 = mybir.dt.bfloat16
    fp8 = mybir.dt.float8e4
    AF = mybir.ActivationFunctionType
    DR = mybir.MatmulPerfMode.DoubleRow

    qT_d = nc.dram_tensor("qT", [P, 2 * KP * PAD_P], fp8,
                          kind="ExternalInput").ap()
    qr_d = nc.dram_tensor("qr", [P, MT * C], bf16, kind="ExternalInput").ap()
    sr_d = nc.dram_tensor("sr", [P, ST * C], bf16, kind="ExternalInput").ap()
    ind_dt = bf16 if SCB else f32
    ind_d = nc.dram_tensor("ind", [P, MT * NQ], ind_dt,
                           kind="ExternalInput").ap()
    ident_d = nc.dram_tensor("ident", [P, P], bf16, kind="ExternalInput").ap()
    out_d = nc.dram_tensor("out", [NQ, WAY], f32, kind="ExternalOutput").ap()

    with tile.TileContext(nc) as tc:
        with ExitStack() as ctx:
            const = ctx.enter_context(tc.tile_pool(name="const", bufs=1))
            big = ctx.enter_context(tc.tile_pool(name="big", bufs=1))
            loads = ctx.enter_context(tc.tile_pool(name="loads", bufs=8))
            work = ctx.enter_context(tc.tile_pool(name="work", bufs=3))
            small = ctx.enter_context(tc.tile_pool(name="small", bufs=4))
            mxp = ctx.enter_context(tc.tile_pool(name="mxp", bufs=MT))
            outp = ctx.enter_context(
                tc.tile_pool(name="outp", bufs=1, space="PSUM")
            )
            tpp = ctx.enter_context(
                tc.tile_pool(name="tpp", bufs=TPP_BUFS, space="PSUM")
            )
            warmp = ctx.enter_context(
                tc.tile_pool(name="warmp", bufs=1, space="PSUM")
            )
            spp = ctx.enter_context(
                tc.tile_pool(name="spp", bufs=3, space="PSUM")
            )

            ident = const.tile([P, P], bf16)
            ind_sb = const.tile([P, MT * NQ], ind_dt)
            # banded transposed operands: [partition, chunk band, column]
            s8T = big.tile([P, 2 * KP, PAD_S], fp8, name="s8T")
            qT_sb = big.tile([P, 2 * KP, PAD_P], fp8, name="qT_sb")
            qr_sb = big.tile([P, MT * C], bf16, name="qr_sb")

            # batched norm scalars
            ssum_all = const.tile([P, ST], f32, name="ssum")
            sinv_all = const.tile([P, ST], f32, name="sinv")
            qsum_all = const.tile([P, MT], f32, name="qsum")
            qinv = const.tile([P, MT], f32, name="qinv")

            out_ps = outp.tile([NQ, WAY], f32)

            # ---- warmups: ACT tables + PE pipeline + pstate ramp ----
            wtile = const.tile([P, P], bf16, name="wtile")
            nc.vector.memset(wtile, 1.0)
            wsq = small.tile([P, P], bf16, tag="wsq")
            wss = small.tile([P, 1], f32, tag="snrm")
            nc.scalar.activation(wsq, wtile, AF.Square, accum_out=wss)
            nc.scalar.sqrt(wss, wss)
            wps = warmp.tile([P, 4 * P], f32, tag="warm")
            for i in range(N_WARM):
                nc.tensor.matmul(
                    wps[:, (i % 4) * P:(i % 4 + 1) * P], wtile, wtile,
                    start=True, stop=True)

            # ---- prologue DMAs ----
            xs_s = [None] * ST

            def s_dma(t):
                x = loads.tile([P, C], bf16, tag="x_tile", name=f"sx{t}")
                nc.sync.dma_start(out=x, in_=sr_d[:, t * C:(t + 1) * C])
                xs_s[t] = x

            for t in range(4):
                s_dma(t)
            nc.sync.dma_start(out=ident, in_=ident_d)
            nc.sync.dma_start(out=qT_sb, in_=qT_d)
            # zero band 5 of s8T on the otherwise-idle gpsimd engine
            nc.gpsimd.memset(s8T[:, 2 * KP - 1, :], 0)
            nc.sync.dma_start(out=ind_sb, in_=ind_d)

            next_s = [4]

            def s_dma_ahead(upto):
                while next_s[0] <= min(upto, ST - 1):
                    s_dma(next_s[0])
                    next_s[0] += 1

            # ---- prep helpers ----
            def s_sq(t, on_dve=False):
                if on_dve and SQ_DVE:
                    sqf = work.tile([P, C], f32, tag="sqf")
                    nc.vector.tensor_tensor_reduce(
                        sqf, xs_s[t], xs_s[t], 1.0, 0.0,
                        mybir.AluOpType.mult, mybir.AluOpType.add,
                        ssum_all[:, t:t + 1])
                else:
                    sq = work.tile([P, C], bf16, tag="sq")
                    nc.scalar.activation(sq, xs_s[t], AF.Square,
                                         accum_out=ssum_all[:, t:t + 1])

            def s_finish(t0, n):
                # tiles t0..t0+n-1: snrm = sqrt(ssum)/alpha ; sinv = alpha/|s|
                sl = slice(t0, t0 + n)
                snrm = small.tile([P, n], f32, tag="snrm", name=f"snrm{t0}")
                nc.scalar.activation(snrm, ssum_all[:, sl], AF.Sqrt,
                                     scale=float(1.0 / (ALPHA * ALPHA)))
                nc.vector.reciprocal(sinv_all[:, sl], snrm)

            def s_mul_transpose_evict(t, mul_on_dve, evict_on_dve):
                x = xs_s[t]
                s_n = work.tile([P, C], bf16, tag="s_n", name=f"sn{t}")
                if mul_on_dve:
                    nc.vector.tensor_scalar_mul(s_n, x, sinv_all[:, t:t + 1])
                else:
                    nc.scalar.mul(s_n, x, sinv_all[:, t:t + 1])
                psA = tpp.tile([P, KC * P], bf16, tag="tp", name=f"psA{t}")
                for c in range(KC):
                    nc.tensor.transpose(
                        psA[:, c * P:(c + 1) * P], s_n[:, c * P:(c + 1) * P],
                        ident)
                dst = s8T[:, 0:KC, t * P:(t + 1) * P]
                src = psA.rearrange("p (c n) -> p c n", c=KC)
                if evict_on_dve:
                    nc.vector.tensor_copy(dst, src)
                else:
                    nc.scalar.copy(dst, src)

            def q_sq(m):
                sq = work.tile([P, C], bf16, tag="sq")
                nc.scalar.activation(sq, qr_sb[:, m * C:(m + 1) * C],
                                     AF.Square,
                                     accum_out=qsum_all[:, m:m + 1])

            # ---- prologue prep: support tiles 0-3 ----
            # pairwise finish so the first transposes start after two squares
            s_dma_ahead(7)
            s_sq(0)
            s_sq(1)
            s_finish(0, 2)
            s_sq(2)
            s_sq(3)
            s_finish(2, 2)
            s_mul_transpose_evict(0, mul_on_dve=True, evict_on_dve=False)
            s_mul_transpose_evict(1, mul_on_dve=True, evict_on_dve=True)
            s_mul_transpose_evict(2, mul_on_dve=True, evict_on_dve=False)
            s_mul_transpose_evict(3, mul_on_dve=True, evict_on_dve=True)
            # query row-major copy is only needed from pass-0 q squares on;
            # chunked DMAs keep prologue HBM bandwidth for the critical
            # support tiles
            def qr_dma(c):
                nc.sync.dma_start(out=qr_sb[:, c * 5 * C:(c + 1) * 5 * C],
                                  in_=qr_d[:, c * 5 * C:(c + 1) * 5 * C])
            qr_dma(0)

            # per-pass schedule slots (pass w<4 preps tiles 4w+4..4w+7):
            # squares early, norm finish, prep units with slack before the
            # pass boundary, q squares interleaved
            SQ_SLOTS = (0, 1, 2, 3)
            UNIT_SLOTS = (5, 7, 9, 11)
            QSQ_SLOTS = (6, 8, 10, 12)

            mxs = [None] * MT
            sc_dt = bf16 if SCB else f32
            prev = [None, None]

            for w in range(WAY):
                for m in range(MT):
                    if w < 4:
                        t_new = 4 * (w + 1)
                        if m == 0:
                            s_dma_ahead(t_new + 7)
                            if w == 0:
                                qr_dma(1)
                        if w == 0 and m == 8:
                            qr_dma(2)
                        if m in SQ_SLOTS:
                            s_sq(t_new + SQ_SLOTS.index(m))
                        if m == 4:
                            s_finish(t_new, 4)
                        if m in UNIT_SLOTS:
                            t = t_new + UNIT_SLOTS.index(m)
                            s_mul_transpose_evict(
                                t, mul_on_dve=(t % 4 == 3),
                                evict_on_dve=(t % 4 == 1))
                        if m in QSQ_SLOTS:
                            mq = 4 * w + QSQ_SLOTS.index(m)
                            if mq < MT:
                                q_sq(mq)
                    if w == 4 and m == 0:
                        # qinv = 1 / (k * alpha * |q_patch|)
                        kn = small.tile([P, MT], f32, tag="kn")
                        nc.scalar.activation(
                            kn, qsum_all, AF.Sqrt,
                            scale=float(k * k * ALPHA * ALPHA))
                        nc.vector.reciprocal(qinv, kn)
                    if w == 0:
                        mxs[m] = mxp.tile([P, WAY * 8], f32, tag="mx",
                                          name=f"mx{m}")
                    psc = spp.tile([P, NW], f32, tag="psc",
                                   name=f"psc{m}_{w}")
                    for j in range(KP):
                        nc.tensor.matmul(
                            psc,
                            qT_sb[:, 2 * j:2 * j + 2, m * P:(m + 1) * P],
                            s8T[:, 2 * j:2 * j + 2, w * NW:(w + 1) * NW],
                            start=(j == 0),
                            stop=(j == KP - 1),
                            perf_mode=DR,
                        )
                    nc.vector.max(mxs[m][:, w * 8:(w + 1) * 8], psc)
                    if w == WAY - 1:
                        tsum = small.tile([P, WAY], f32, tag="tsum")
                        nc.vector.tensor_reduce(
                            tsum,
                            mxs[m].rearrange("p (w j) -> p w j", w=WAY)[:, :, :k],
                            axis=mybir.AxisListType.X,
                            op=mybir.AluOpType.add,
                        )
                        # ACT is otherwise idle in the final pass
                        scaled = small.tile([P, WAY], sc_dt, tag="scaled")
                        nc.scalar.mul(scaled, tsum, qinv[:, m:m + 1])
                        if prev[0] is not None:
                            nc.tensor.matmul(
                                out_ps,
                                ind_sb[:, prev[1] * NQ:(prev[1] + 1) * NQ],
                                prev[0], start=(prev[1] == 0), stop=False)
                        prev[0], prev[1] = scaled, m
            nc.tensor.matmul(
                out_ps, ind_sb[:, prev[1] * NQ:(prev[1] + 1) * NQ],
                prev[0], start=False, stop=True)
            out_sb = small.tile([NQ, WAY], f32, tag="out_sb")
            nc.vector.tensor_copy(out_sb, out_ps)
            nc.sync.dma_start(out=out_d, in_=out_sb)

    nc.compile()
    return nc


def get_program(k: int):
    if k not in _prog_cache:
        _prog_cache[k] = _build(k)
    return _prog_cache[k]


def make_in_maps(input1: np.ndarray, input2: np.ndarray):
    """Shard full inputs into per-core input maps (layout + dtype only)."""
    input1 = np.ascontiguousarray(np.asarray(input1), dtype=np.float32)
    input2 = np.ascontiguousarray(np.asarray(input2), dtype=np.float32)
    fp8 = ml_dtypes.float8_e4m3fn
    bf16 = ml_dtypes.bfloat16
    in_maps = []
    for core in range(N_CORES):
        b = core // 4
        qs = (core % 4) * NQ
        qe = min(Q, qs + NQ)
        nq = qe - qs
        qdat = input1[b].reshape(Q, HW, C)[qs:qe].reshape(-1, C)
        qfull = np.ones((PAD_P, C), np.float32)
        qfull[: nq * HW] = qdat
        # qT: fp8 banded transpose [128, 6, PAD_P] (band 5 zeros)
        q8 = qfull.astype(fp8)
        qT = np.zeros((P, 2 * KP, PAD_P), fp8)
        for c in range(KC):
            qT[:, c, :] = q8[:, c * P:(c + 1) * P].T
        # q row-major bf16, partition-major tiles [128, MT, C]
        qr = np.ascontiguousarray(
            qfull.reshape(MT, P, C).transpose(1, 0, 2)).astype(bf16)
        sfull = np.ones((PAD_S, C), np.float32)
        sfull[:NS] = input2[b].reshape(NS, C)
        sr = np.ascontiguousarray(
            sfull.reshape(ST, P, C).transpose(1, 0, 2)).astype(bf16)
        # indicator: patch row p of M-tile t belongs to query (t*128+p)//HW
        ind = np.zeros((P, MT * NQ), np.float32)
        g = np.arange(MT * P)
        j = g // HW
        valid = j < nq
        ind[g[valid] % P, (g[valid] // P) * NQ + j[valid]] = 1.0
        in_maps.append({
            "qT": qT.reshape(P, 2 * KP * PAD_P),
            "qr": qr.reshape(P, MT * C),
            "sr": sr.reshape(P, ST * C),
            "ind": ind.astype(bf16) if SCB else ind,
            "ident": np.eye(P).astype(bf16),
        })
    return in_maps


def gather_out(results) -> np.ndarray:
    out = np.zeros((B, Q, WAY), np.float32)
    for core in range(N_CORES):
        b = core // 4
        qs = (core % 4) * NQ
        n = min(Q, qs + NQ) - qs
        out[b, qs:qs + n] = results[core]["out"][:n]
    return out


def kernel(input1, input2, neighbor_k):
    k = int(np.asarray(neighbor_k))
    nc = get_program(k)
    in_maps = make_in_maps(input1, input2)
    # the axon-tunneled device occasionally reports a transient
    # "unrecoverable" state right after a previous process's teardown;
    # it recovers within seconds, so retry a couple of times
    import time
    last = None
    for attempt in range(3):
        try:
            res = run_bass_kernel_spmd(
                nc, in_maps, core_ids=list(range(N_CORES)))
            return gather_out(res.results)
        except Exception as e:  # noqa: BLE001
            last = e
            if attempt < 2:
                time.sleep(20.0 * (attempt + 1))
    raise last


# revision 27
# speedup vs baseline: 1.1029x; 1.0142x over previous
"""TRN2 Bass kernel for nn_MetaBaseline (DN4-style local-descriptor kNN).

Reference computation (per batch b):
  q = input1[b].reshape(7500, 640)           # query patch descriptors
  s = normalize(input2[b].reshape(2500, 640), axis=-1)
  scores = q @ s.T                           # [7500, 2500]
  per way group g (columns [500g, 500g+500)): top-k per row, mean,
  divide by |q_patch| (commutes with top-k since it is per-row), then
  sum over the 100 patches of each query -> out [75, 5].

Sharding: data-parallel over (b, query-quarter): 8 cores, each handles one
batch's quarter of queries (19 queries padded) with that batch's full
support replicated.

Implementation highlights vs the fp32r baseline:
- Score matmuls run in fp8e4m3 with MatmulPerfMode.DoubleRow (two 128-row
  k-tiles per pass): C=640 is padded to 6 chunks of 128 (band 5 zeros) so
  each (patch-tile, way) pair needs just 3 PE instructions.
- The query operand is uploaded pre-transposed in fp8 from the host
  (layout + dtype conversion only), so queries need no PE transpose, no
  PSUM eviction and no quantize pass on device. A second row-major bf16
  copy of q feeds the |q_patch| norm pass (ACT square+accum), matching the
  reference's exact-norm division.
- Support arrives row-major bf16; on device: ACT square+accum -> batched
  sqrt(scale=1/alpha^2) -> DVE reciprocal gives sinv = alpha/|s| per
  descriptor; DVE tensor_scalar (4x mode on bf16) or ACT scales to
  alpha-normalized bf16; PE transposes (bf16 identity, 1 cycle/row) into
  one PSUM bank per tile; a single strided copy evicts+converts to the
  fp8 banded layout. alpha=32 keeps e4m3 operands in the normal range;
  1/alpha folds into the per-query scale sqrt(k^2 alpha^2 sum q^2).
- Top-8 per (patch, way) via DVE max8 straight from the PSUM score bank;
  pass 4 finishes each patch tile with a strided top-k tensor_reduce, an
  ACT scale by 1/(k*alpha*|q_patch|), and a small fp32 indicator matmul
  accumulating per-query sums in PSUM -> [19, 5].
"""
import os
from contextlib import ExitStack

import ml_dtypes
import numpy as np

import concourse.bass as bass  # noqa: F401
import concourse.mybir as mybir
import concourse.tile as tile
from concourse import bacc
from concourse.bass_utils import run_bass_kernel_spmd

# Problem geometry (hardcoded per contest rules)
B, Q, WAY, SHOT, H, W, C = 2, 75, 5, 5, 10, 10, 640
HW = H * W               # 100 patches per query / support image
NQ = 19                  # queries per core (4 cores x 19 = 76 >= 75)
MT = 15                  # patch M-tiles of 128 -> 1920 rows (1900 real)
PAD_P = MT * 128
NS = WAY * SHOT * HW     # 2500 support descriptors per batch
ST = 20                  # support tiles of 128 -> 2560 rows
PAD_S = ST * 128
KC = 5                   # real C chunks of 128 (640 = 5*128)
KP = 3                   # DoubleRow chunk pairs (6 bands incl. zero band)
P = 128
NW = SHOT * HW           # 500 support descriptors per way group
N_CORES = 8
ALPHA = 32.0             # support operand scale for fp8 normal range
N_WARM = int(os.environ.get("N_WARM", "58"))
# NOTE: tensor_tensor_reduce with bf16 inputs faults on real TRN2 hardware
# (verified by bisection); squares therefore run on ACT only.
SQ_DVE = os.environ.get("SQ_DVE", "0") == "1"   # squares split ACT/DVE
TPP_BUFS = int(os.environ.get("TPP", "3"))       # transpose PSUM banks
SCB = os.environ.get("SCB", "1") == "1"          # scaled on DVE bf16 + bf16 ind

_prog_cache: dict[int, object] = {}


def _build(k: int):
    """Build + compile the per-core SPMD program for neighbor_k == k."""
    assert 1 <= k <= 8, f"neighbor_k={k} not supported (need 1..8)"
    nc = bacc.Bacc("TRN2", target_bir_lowering=False, debug=False)
    f32 = mybir.dt.float32
    bf16 = mybir.dt.bfloat16
    fp8 = mybir.dt.float8e4
    AF = mybir.ActivationFunctionType
    DR = mybir.MatmulPerfMode.DoubleRow

    qT_d = nc.dram_tensor("qT", [P, 2 * KP * PAD_P], fp8,
                          kind="ExternalInput").ap()
    qr_d = nc.dram_tensor("qr", [P, MT * C], bf16, kind="ExternalInput").ap()
    sr_d = nc.dram_tensor("sr", [P, ST * C], bf16, kind="ExternalInput").ap()
    ind_dt = bf16 if SCB else f32
    ind_d = nc.dram_tensor("ind", [P, MT * NQ], ind_dt,
                           kind="ExternalInput").ap()
    ident_d = nc.dram_tensor("ident", [P, P], bf16, kind="ExternalInput").ap()
    out_d = nc.dram_tensor("out", [NQ, WAY], f32, kind="ExternalOutput").ap()

    with tile.TileContext(nc) as tc:
        with ExitStack() as ctx:
            const = ctx.enter_context(tc.tile_pool(name="const", bufs=1))
            big = ctx.enter_context(tc.tile_pool(name="big", bufs=1))
            loads = ctx.enter_context(tc.tile_pool(name="loads", bufs=8))
            work = ctx.enter_context(tc.tile_pool(name="work", bufs=3))
            small = ctx.enter_context(tc.tile_pool(name="small", bufs=4))
            mxp = ctx.enter_context(tc.tile_pool(name="mxp", bufs=MT))
            outp = ctx.enter_context(
                tc.tile_pool(name="outp", bufs=1, space="PSUM")
            )
            tpp = ctx.enter_context(
                tc.tile_pool(name="tpp", bufs=TPP_BUFS, space="PSUM")
            )
            warmp = ctx.enter_context(
                tc.tile_pool(name="warmp", bufs=1, space="PSUM")
            )
            spp = ctx.enter_context(
                tc.tile_pool(name="spp", bufs=3, space="PSUM")
            )

            ident = const.tile([P, P], bf16)
            ind_sb = const.tile([P, MT * NQ], ind_dt)
            # banded transposed operands: [partition, chunk band, column]
            s8T = big.tile([P, 2 * KP, PAD_S], fp8, name="s8T")
            qT_sb = big.tile([P, 2 * KP, PAD_P], fp8, name="qT_sb")
            qr_sb = big.tile([P, MT * C], bf16, name="qr_sb")

            # batched norm scalars
            ssum_all = const.tile([P, ST], f32, name="ssum")
            sinv_all = const.tile([P, ST], f32, name="sinv")
            qsum_all = const.tile([P, MT], f32, name="qsum")
            qinv = const.tile([P, MT], f32, name="qinv")

            out_ps = outp.tile([NQ, WAY], f32)

            # ---- warmups: ACT tables + PE pipeline + pstate ramp ----
            wtile = const.tile([P, P], bf16, name="wtile")
            nc.vector.memset(wtile, 1.0)
            wsq = small.tile([P, P], bf16, tag="wsq")
            wss = small.tile([P, 1], f32, tag="snrm")
            nc.scalar.activation(wsq, wtile, AF.Square, accum_out=wss)
            nc.scalar.sqrt(wss, wss)
            wps = warmp.tile([P, 4 * P], f32, tag="warm")
            for i in range(N_WARM):
                nc.tensor.matmul(
                    wps[:, (i % 4) * P:(i % 4 + 1) * P], wtile, wtile,
                    start=True, stop=True)

            # ---- prologue DMAs ----
            xs_s = [None] * ST

            def s_dma(t):
                x = loads.tile([P, C], bf16, tag="x_tile", name=f"sx{t}")
                nc.sync.dma_start(out=x, in_=sr_d[:, t * C:(t + 1) * C])
                xs_s[t] = x

            for t in range(4):
                s_dma(t)

            def qT_dma(j):
                nc.sync.dma_start(
                    out=qT_sb[:, 2 * j:2 * j + 2, :],
                    in_=qT_d[:, 2 * j * PAD_P:(2 * j + 2) * PAD_P])

            # first score group only needs chunk-pair 0; later pairs stream
            # behind the critical support tiles
            qT_dma(0)
            nc.sync.dma_start(out=ident, in_=ident_d)
            nc.sync.dma_start(out=ind_sb, in_=ind_d)
            qT_dma(1)
            qT_dma(2)
            # zero band 5 of s8T on the otherwise-idle gpsimd engine
            nc.gpsimd.memset(s8T[:, 2 * KP - 1, :], 0)

            next_s = [4]

            def s_dma_ahead(upto):
                while next_s[0] <= min(upto, ST - 1):
                    s_dma(next_s[0])
                    next_s[0] += 1

            # ---- prep helpers ----
            def s_sq(t, on_dve=False):
                if on_dve and SQ_DVE:
                    sqf = work.tile([P, C], f32, tag="sqf")
                    nc.vector.tensor_tensor_reduce(
                        sqf, xs_s[t], xs_s[t], 1.0, 0.0,
                        mybir.AluOpType.mult, mybir.AluOpType.add,
                        ssum_all[:, t:t + 1])
                else:
                    sq = work.tile([P, C], bf16, tag="sq")
                    nc.scalar.activation(sq, xs_s[t], AF.Square,
                                         accum_out=ssum_all[:, t:t + 1])

            def s_finish(t0, n):
                # tiles t0..t0+n-1: snrm = sqrt(ssum)/alpha ; sinv = alpha/|s|
                sl = slice(t0, t0 + n)
                snrm = small.tile([P, n], f32, tag="snrm", name=f"snrm{t0}")
                nc.scalar.activation(snrm, ssum_all[:, sl], AF.Sqrt,
                                     scale=float(1.0 / (ALPHA * ALPHA)))
                nc.vector.reciprocal(sinv_all[:, sl], snrm)

            def s_mul_transpose_evict(t, mul_on_dve, evict_on_dve):
                x = xs_s[t]
                s_n = work.tile([P, C], bf16, tag="s_n", name=f"sn{t}")
                if mul_on_dve:
                    nc.vector.tensor_scalar_mul(s_n, x, sinv_all[:, t:t + 1])
                else:
                    nc.scalar.mul(s_n, x, sinv_all[:, t:t + 1])
                psA = tpp.tile([P, KC * P], bf16, tag="tp", name=f"psA{t}")
                for c in range(KC):
                    nc.tensor.transpose(
                        psA[:, c * P:(c + 1) * P], s_n[:, c * P:(c + 1) * P],
                        ident)
                dst = s8T[:, 0:KC, t * P:(t + 1) * P]
                src = psA.rearrange("p (c n) -> p c n", c=KC)
                if evict_on_dve:
                    nc.vector.tensor_copy(dst, src)
                else:
                    nc.scalar.copy(dst, src)

            def q_sq(m):
                sq = work.tile([P, C], bf16, tag="sq")
                nc.scalar.activation(sq, qr_sb[:, m * C:(m + 1) * C],
                                     AF.Square,
                                     accum_out=qsum_all[:, m:m + 1])

            # ---- prologue prep: support tiles 0-3 ----
            # pairwise finish so the first transposes start after two squares
            s_dma_ahead(7)
            s_sq(0)
            s_sq(1)
            s_finish(0, 2)
            s_sq(2)
            s_sq(3)
            s_finish(2, 2)
            s_mul_transpose_evict(0, mul_on_dve=True, evict_on_dve=False)
            s_mul_transpose_evict(1, mul_on_dve=True, evict_on_dve=True)
            s_mul_transpose_evict(2, mul_on_dve=True, evict_on_dve=False)
            s_mul_transpose_evict(3, mul_on_dve=True, evict_on_dve=True)
            # query row-major copy is only needed from pass-0 q squares on;
            # chunked DMAs keep prologue HBM bandwidth for the critical
            # support tiles
            def qr_dma(c):
                nc.sync.dma_start(out=qr_sb[:, c * 5 * C:(c + 1) * 5 * C],
                                  in_=qr_d[:, c * 5 * C:(c + 1) * 5 * C])
            qr_dma(0)

            # per-pass schedule slots (pass w<4 preps tiles 4w+4..4w+7):
            # squares early, norm finish, prep units with slack before the
            # pass boundary, q squares interleaved
            SQ_SLOTS = (0, 1, 2, 3)
            UNIT_SLOTS = (5, 7, 9, 11)
            QSQ_SLOTS = (6, 8, 10, 12)

            mxs = [None] * MT
            sc_dt = bf16 if SCB else f32
            prev = [None, None]

            for w in range(WAY):
                for m in range(MT):
                    if w < 4:
                        t_new = 4 * (w + 1)
                        if m == 0:
                            s_dma_ahead(t_new + 7)
                            if w == 0:
                                qr_dma(1)
                        if w == 0 and m == 8:
                            qr_dma(2)
                        if m in SQ_SLOTS:
                            s_sq(t_new + SQ_SLOTS.index(m))
                        if m == 4:
                            s_finish(t_new, 4)
                        if m in UNIT_SLOTS:
                            t = t_new + UNIT_SLOTS.index(m)
                            s_mul_transpose_evict(
                                t, mul_on_dve=(t % 4 == 3),
                                evict_on_dve=(t % 4 == 1))
                        if m in QSQ_SLOTS:
                            mq = 4 * w + QSQ_SLOTS.index(m)
                            if mq < MT:
                                q_sq(mq)
                    if w == 4 and m == 0:
                        # qinv = 1 / (k * alpha * |q_patch|)
                        kn = small.tile([P, MT], f32, tag="kn")
                        nc.scalar.activation(
                            kn, qsum_all, AF.Sqrt,
                            scale=float(k * k * ALPHA * ALPHA))
                        nc.vector.reciprocal(qinv, kn)
                    if w == 0:
                        mxs[m] = mxp.tile([P, WAY * 8], f32, tag="mx",
                                          name=f"mx{m}")
                    psc = spp.tile([P, NW], f32, tag="psc",
                                   name=f"psc{m}_{w}")
                    for j in range(KP):
                        nc.tensor.matmul(
                            psc,
                            qT_sb[:, 2 * j:2 * j + 2, m * P:(m + 1) * P],
                            s8T[:, 2 * j:2 * j + 2, w * NW:(w + 1) * NW],
                            start=(j == 0),
                            stop=(j == KP - 1),
                            perf_mode=DR,
                        )
                    nc.vector.max(mxs[m][:, w * 8:(w + 1) * 8], psc)
                    if w == WAY - 1:
                        tsum = small.tile([P, WAY], f32, tag="tsum")
                        nc.vector.tensor_reduce(
                            tsum,
                            mxs[m].rearrange("p (w j) -> p w j", w=WAY)[:, :, :k],
                            axis=mybir.AxisListType.X,
                            op=mybir.AluOpType.add,
                        )
                        # ACT is otherwise idle in the final pass
                        scaled = small.tile([P, WAY], sc_dt, tag="scaled")
                        nc.scalar.mul(scaled, tsum, qinv[:, m:m + 1])
                        if prev[0] is not None:
                            nc.tensor.matmul(
                                out_ps,
                                ind_sb[:, prev[1] * NQ:(prev[1] + 1) * NQ],
                                prev[0], start=(prev[1] == 0), stop=False)
                        prev[0], prev[1] = scaled, m
            nc.tensor.matmul(
                out_ps, ind_sb[:, prev[1] * NQ:(prev[1] + 1) * NQ],
                prev[0], start=False, stop=True)
            out_sb = small.tile([NQ, WAY], f32, tag="out_sb")
            nc.vector.tensor_copy(out_sb, out_ps)
            nc.sync.dma_start(out=out_d, in_=out_sb)

    nc.compile()
    return nc


def get_program(k: int):
    if k not in _prog_cache:
        _prog_cache[k] = _build(k)
    return _prog_cache[k]


def make_in_maps(input1: np.ndarray, input2: np.ndarray):
    """Shard full inputs into per-core input maps (layout + dtype only)."""
    input1 = np.ascontiguousarray(np.asarray(input1), dtype=np.float32)
    input2 = np.ascontiguousarray(np.asarray(input2), dtype=np.float32)
    fp8 = ml_dtypes.float8_e4m3fn
    bf16 = ml_dtypes.bfloat16
    in_maps = []
    for core in range(N_CORES):
        b = core // 4
        qs = (core % 4) * NQ
        qe = min(Q, qs + NQ)
        nq = qe - qs
        qdat = input1[b].reshape(Q, HW, C)[qs:qe].reshape(-1, C)
        qfull = np.ones((PAD_P, C), np.float32)
        qfull[: nq * HW] = qdat
        # qT: fp8 banded transpose [128, 6, PAD_P] (band 5 zeros)
        q8 = qfull.astype(fp8)
        qT = np.zeros((P, 2 * KP, PAD_P), fp8)
        for c in range(KC):
            qT[:, c, :] = q8[:, c * P:(c + 1) * P].T
        # q row-major bf16, partition-major tiles [128, MT, C]
        qr = np.ascontiguousarray(
            qfull.reshape(MT, P, C).transpose(1, 0, 2)).astype(bf16)
        sfull = np.ones((PAD_S, C), np.float32)
        sfull[:NS] = input2[b].reshape(NS, C)
        sr = np.ascontiguousarray(
            sfull.reshape(ST, P, C).transpose(1, 0, 2)).astype(bf16)
        # indicator: patch row p of M-tile t belongs to query (t*128+p)//HW
        ind = np.zeros((P, MT * NQ), np.float32)
        g = np.arange(MT * P)
        j = g // HW
        valid = j < nq
        ind[g[valid] % P, (g[valid] // P) * NQ + j[valid]] = 1.0
        in_maps.append({
            "qT": qT.reshape(P, 2 * KP * PAD_P),
            "qr": qr.reshape(P, MT * C),
            "sr": sr.reshape(P, ST * C),
            "ind": ind.astype(bf16) if SCB else ind,
            "ident": np.eye(P).astype(bf16),
        })
    return in_maps


def gather_out(results) -> np.ndarray:
    out = np.zeros((B, Q, WAY), np.float32)
    for core in range(N_CORES):
        b = core // 4
        qs = (core % 4) * NQ
        n = min(Q, qs + NQ) - qs
        out[b, qs:qs + n] = results[core]["out"][:n]
    return out


def kernel(input1, input2, neighbor_k):
    k = int(np.asarray(neighbor_k))
    nc = get_program(k)
    in_maps = make_in_maps(input1, input2)
    # the axon-tunneled device occasionally reports a transient
    # "unrecoverable" state right after a previous process's teardown;
    # it recovers within seconds, so retry a couple of times
    import time
    last = None
    for attempt in range(3):
        try:
            res = run_bass_kernel_spmd(
                nc, in_maps, core_ids=list(range(N_CORES)))
            return gather_out(res.results)
        except Exception as e:  # noqa: BLE001
            last = e
            if attempt < 2:
                time.sleep(20.0 * (attempt + 1))
    raise last
